# revision 18
# baseline (speedup 1.0000x reference)
"""RNN-T joint network kernel for Trainium2 (Bass/Tile), 8-core data-parallel.

Problem: out[b,t,u,:] = tanh(enc[b,t]@W_enc + b_enc + dec[b,u]@W_dec + b_dec) @ W_out + b_out
Shapes: B=8, T=256, U=64, D=512, J=640, V=1024 (all fp32).

Sharding: data-parallel over batch B across the 8 NeuronCores (1 batch element
per core). Per core the dominant work is the joint matmul (T,U,J)x(J,V):
1280 bf16 matmuls of N=512 -> ~273us at the 2.4 GHz PE clock (the whole-chip
power state sometimes throttles all engines to 83%, ~332us); the 64MB fp32
output DMA and the tanh/drain engines fit underneath the PE time.

Per-core plan (all J-major layouts so J is the matmul contraction partition dim):
  host:   inputs pre-transposed AND pre-packed per 128-row chunk into single
          [128, n_chunks*width] arrays so every constant loads with ONE
          contiguous DMA descriptor (setup head is DMA-issue-rate bound).
          Projection operands in bf16 (single-pass matmuls; fp32 would lower
          to LOW/HIGH double-pass and double the setup time).
  setup:  enc_projT[j,t] = W_enc^T @ encT (bf16 mms, fp32 PSUM), dec_projT
          likewise with (b_enc+b_dec) folded in via ACT bias on the drain.
  main:   per u: hT[j,t] = tanh(enc_projT[j,t] + dec_projT[j,u]) via ACT
          (bias = per-partition dec column); joint matmul in bf16 into one
          PSUM bank per (t-tile, v-half) group of 5 mms, 8-bank ring, so each
          bank recycles after a single 512-col DVE drain (add broadcast b_out
          PSUM->SBUF); 256KB output DMA per group on alternating queues.
"""

import numpy as np
from contextlib import ExitStack

from concourse import bacc, bass, tile
from concourse.bass import mybir
from concourse.bass_utils import run_bass_kernel_spmd

F32 = mybir.dt.float32
BF16 = mybir.dt.bfloat16
ACT_F = mybir.ActivationFunctionType

B, T, U = 8, 256, 64
D, J, V = 512, 640, 1024
NJC = J // 128   # 5 contraction chunks of the joint matmul
NDC = D // 128   # 4 contraction chunks of the projections
NVB = V // 512   # 2 v-halves (one psum bank each) per joint output tile


def build_program() -> bass.Bass:
    nc = bacc.Bacc("TRN2", target_bir_lowering=False, debug=False)

    # packed layouts: pk[p, c*W + x] = orig[c*128 + p, x]
    encT_d = nc.declare_dram_parameter("encT", [128, NDC * T], BF16, isOutput=False)
    decT_d = nc.declare_dram_parameter("decT", [128, NDC * U], BF16, isOutput=False)
    w_enc_d = nc.declare_dram_parameter("w_enc", [128, NDC * J], BF16, isOutput=False)
    w_dec_d = nc.declare_dram_parameter("w_dec", [128, NDC * J], BF16, isOutput=False)
    bb_d = nc.declare_dram_parameter("bb", [128, NJC], F32, isOutput=False)  # b_enc+b_dec
    w_out_d = nc.declare_dram_parameter("w_out", [128, NJC * V], BF16, isOutput=False)
    b_out_d = nc.declare_dram_parameter("b_out", [V], F32, isOutput=False)
    out = nc.declare_dram_parameter("out", [T, U, V], F32, isOutput=True)

    with tile.TileContext(nc) as tc, ExitStack() as ctx:
        const = ctx.enter_context(tc.tile_pool(name="const", bufs=1))

        # --- resident constants, issued in dependency-priority order --------
        # (all on the sync HW-DGE queue: the gpsimd SW-DGE queue transfers
        # bulk weights measurably slower and delays the projections)
        bbt = const.tile([128, NJC], F32)
        nc.sync.dma_start(out=bbt[:], in_=bb_d[:])
        decT = const.tile([128, NDC * U], BF16)
        nc.sync.dma_start(out=decT[:], in_=decT_d[:])
        w_dec_sb = const.tile([128, NDC * J], BF16)
        nc.sync.dma_start(out=w_dec_sb[:], in_=w_dec_d[:])
        encT = const.tile([128, NDC * T], BF16)
        nc.sync.dma_start(out=encT[:], in_=encT_d[:])
        w_enc_sb = const.tile([128, NDC * J], BF16)
        nc.sync.dma_start(out=w_enc_sb[:], in_=w_enc_d[:])
        w_out_sb = const.tile([128, NJC * V], BF16)
        nc.sync.dma_start(out=w_out_sb[:], in_=w_out_d[:])
        bias_rep = const.tile([128, V], F32)
        nc.gpsimd.dma_start(
            out=bias_rep[:],
            in_=b_out_d[:].unsqueeze(0).broadcast_to((128, V)),
        )

        enc_projT = [const.tile([128, T], F32, name=f"ep{jc}") for jc in range(NJC)]
        dec_projT = [const.tile([128, U], F32, name=f"dp{jc}") for jc in range(NJC)]

        # --- setup: input projections (bf16 mms, fp32 accumulation) ---------
        with tc.tile_pool(name="setup_ps", bufs=4, space="PSUM") as setup_ps:
            for jc in range(NJC):
                ps = setup_ps.tile([128, U], F32, tag="dproj")
                for dc in range(NDC):
                    nc.tensor.matmul(
                        ps[:],
                        w_dec_sb[:, dc * J + jc * 128 : dc * J + (jc + 1) * 128],
                        decT[:, dc * U : (dc + 1) * U],
                        start=(dc == 0),
                        stop=(dc == NDC - 1),
                    )
                # fold b_enc+b_dec into dec_projT during the PSUM->SBUF drain
                nc.scalar.activation(
                    dec_projT[jc][:], ps[:], ACT_F.Identity,
                    bias=bbt[:, jc : jc + 1], scale=1.0,
                )

                ps = setup_ps.tile([128, T], F32, tag="eproj")
                for dc in range(NDC):
                    nc.tensor.matmul(
                        ps[:],
                        w_enc_sb[:, dc * J + jc * 128 : dc * J + (jc + 1) * 128],
                        encT[:, dc * T : (dc + 1) * T],
                        start=(dc == 0),
                        stop=(dc == NDC - 1),
                    )
                nc.vector.tensor_copy(enc_projT[jc][:], ps[:])

        # --- main loop, one u per iteration ---------------------------------
        # Each hT[jc] tile is fully consumed by 20 consecutive MMs (both
        # t-tiles, both v-halves), so its pool slot recycles in ~4.3us and
        # the ACT tanh stream can run several u ahead of the PE.
        h_pool = ctx.enter_context(tc.tile_pool(name="h", bufs=6))
        st_pool = ctx.enter_context(tc.tile_pool(name="stage", bufs=1))
        mm_ps = ctx.enter_context(tc.tile_pool(name="mm_ps", bufs=1, space="PSUM"))
        NST = 12  # stage ring depth (explicit round-robin tags)

        def drain_and_store(ps, u, tt, vv, g):
            # drain PSUM -> SBUF while adding the broadcast b_out, then DMA out
            stage = st_pool.tile([128, 1, 512], F32, tag=f"st{g % NST}", name="stage")
            nc.vector.tensor_add(
                stage[:, 0, :],
                ps[:],
                bias_rep[:, vv * 512 : (vv + 1) * 512],
            )
            q = nc.sync if (tt + vv) % 2 == 0 else nc.gpsimd
            q.dma_start(
                out=out[
                    tt * 128 : (tt + 1) * 128,
                    u : u + 1,
                    vv * 512 : (vv + 1) * 512,
                ],
                in_=stage[:],
            )

        pend = None  # previous group's (ps, u, tt, vv, g), drained one group late
        for u in range(U):
            hT = [h_pool.tile([128, T], BF16, tag=f"h{jc}", name=f"h{jc}") for jc in range(NJC)]
            for jc in range(NJC):
                nc.scalar.activation(
                    hT[jc][:],
                    enc_projT[jc][:],
                    ACT_F.Tanh,
                    bias=dec_projT[jc][:, u : u + 1],
                    scale=1.0,
                )
            # one PSUM bank per (tt,vv) group: each drain's RAW is only its own
            # 5 mms and each bank recycles after a single 512-col drain. The
            # drain is EMITTED one group late so the scheduler's group-start /
            # drain pairing carries an extra group of margin (the steady-state
            # margin is otherwise ~190ns and periodic jitter costs a mm slot).
            for tt in range(T // 128):
                for vv in range(NVB):
                    g = u * 4 + tt * 2 + vv
                    ps = mm_ps.tile([128, 512], F32, tag=f"mm{g % 8}", name="ps")
                    for jc in range(NJC):
                        nc.tensor.matmul(
                            ps[:],
                            hT[jc][:, tt * 128 : (tt + 1) * 128],
                            w_out_sb[:, jc * V + vv * 512 : jc * V + (vv + 1) * 512],
                            start=(jc == 0),
                            stop=(jc == NJC - 1),
                        )
                    if pend is not None:
                        drain_and_store(*pend)
                    pend = (ps, u, tt, vv, g)
        drain_and_store(*pend)

    nc.finalize()
    return nc


_PROGRAM = None


def _pack(a: np.ndarray, nchunk: int) -> np.ndarray:
    """[nchunk*128, W] -> [128, nchunk*W] with pk[p, c*W+x] = a[c*128+p, x]."""
    w = a.shape[1]
    return np.ascontiguousarray(
        a.reshape(nchunk, 128, w).transpose(1, 0, 2).reshape(128, nchunk * w)
    )


def _make_in_maps(enc_out, dec_out, W_enc, b_enc, W_dec, b_dec, W_out, b_out):
    import ml_dtypes

    bf16 = ml_dtypes.bfloat16
    bb = (np.asarray(b_enc, np.float32) + np.asarray(b_dec, np.float32))
    bb_pk = np.ascontiguousarray(bb.reshape(NJC, 128).T)
    w_enc_pk = _pack(np.asarray(W_enc, np.float32), NDC).astype(bf16)
    w_dec_pk = _pack(np.asarray(W_dec, np.float32), NDC).astype(bf16)
    w_out_pk = _pack(np.asarray(W_out, np.float32), NJC).astype(bf16)
    b_out_f = np.asarray(b_out, np.float32)
    enc_f = np.asarray(enc_out, np.float32)
    dec_f = np.asarray(dec_out, np.float32)

    in_maps = []
    for b in range(B):
        in_maps.append(
            {
                "encT": _pack(np.ascontiguousarray(enc_f[b, :, 0, :].T), NDC).astype(bf16),
                "decT": _pack(np.ascontiguousarray(dec_f[b, 0, :, :].T), NDC).astype(bf16),
                "w_enc": w_enc_pk,
                "w_dec": w_dec_pk,
                "bb": bb_pk,
                "w_out": w_out_pk,
                "b_out": b_out_f,
            }
        )
    return in_maps


def kernel(enc_out, dec_out, W_enc, b_enc, W_dec, b_dec, W_out, b_out):
    global _PROGRAM
    if _PROGRAM is None:
        _PROGRAM = build_program()

    in_maps = _make_in_maps(
        enc_out, dec_out, W_enc, b_enc, W_dec, b_dec, W_out, b_out
    )
    res = run_bass_kernel_spmd(_PROGRAM, in_maps, list(range(B)))
    return np.stack([res.results[b]["out"] for b in range(B)], axis=0)
